# revision 8
# baseline (speedup 1.0000x reference)
"""Trainium2 Bass kernel for nn_LinearEncoder (gnn_message_passing).

Reference computes, for N=512 nodes with n_in = n_out = 256:
    i, j = triu_indices(N, k=1)
    edges = concat([x[i], x[j]], -1)            # [E, 512]
    h = edges @ W.T + b                         # [E, n_out]
    out[i, j] = h ; out = out + out.T           # [N, N, 256], 0 diagonal

Key identities: with W = [W1 | W2], A = x @ W1.T, B' = x @ W2.T + b,
the full output is symmetric with zero diagonal and, on the upper
triangle (i < j),
    out[i, j, c] = A[i, c] + B'[j, c].
The device therefore only materialises the strict upper triangle; the
host's unshard step places each value at both (i, j) and (j, i) (the
diagonal stays at the scatter-init zero), halving the HBM write stream
versus a full-matrix kernel.

Layout: channels on SBUF partitions (two 128-channel halves), nodes on
the free dimension.  The column tables B'T[c, j] fall straight out of
two K=256 matmuls against the uploaded x.T; the per-row terms A[i, c]
are [128, 1] columns of RS = W1 @ x[rows_k].T (host slices the 64
owned rows per core), so each output row segment is a single
per-partition-scalar add: out_seg = B'T[:, j0:512] + RS[:, m].

Sharding: core k owns rows i = 32*b + 4*k + v (b in [0,16), v in
[0,4)) — four rows from every 32-row column block, so each core's
upper-triangle rectangles (cols [32b, 512) for block b) have identical
shapes across cores (one SPMD program) and identical total bytes.
Blocks b and 15-b pair into eight [128, 4352] bf16 slabs (1.09 MB
HWDGE DMAs, ~8.9 MB/core total).  Sub-diagonal lanes inside a
rectangle are shipped as garbage and discarded by the host (the mirror
of the transposed upper triangle supplies those entries).

Startup latency management: all inputs arrive as two packed [128, 1092]
bf16 tiles (one HWDGE DMA each, back-to-back on the sync queue; the two
f32 bias columns ride along via a bf16 bitcast), the in-chunk-0 matmuls
run while chunk 1 is still in flight, and slab 0 ships as two 544 KB
halves with its v0/v1 ops front-loaded under high priority.

Engine assignment is calibrated to measured TRN2 op costs: DVE
tensor_scalar (4x uop, ~165 ns overhead + 0.26 ns/col) takes the wide
segments, ScalarE ACT-with-bias (~187 + 0.83/col) the mid ones, and
the narrow block-B segments run as one fused tensor_tensor per half
with stride-0/stride-1 broadcast APs over the four v-rows (GpSimd for
p in 1..4, DVE for p=5).  GpSimd tensor_scalar is never used: it runs
~7x slower than its tensor_tensor and its SBUF traffic starves
concurrent DVE ops.
"""

import contextlib
import os
import sys

for _p in ("/opt/trn_rl_repo", "/root/.axon_site/_ro/trn_rl_repo"):
    if os.path.isdir(_p) and _p not in sys.path:
        sys.path.insert(0, _p)

import numpy as np
import ml_dtypes

import concourse.bass as bass
import concourse.bacc as bacc
import concourse.mybir as mybir
import concourse.tile as tile
from concourse.bass_utils import run_bass_kernel_spmd

N = 512
CH = 256          # n_out
NIN = 256         # n_in
NCORES = 8
NB = 16           # column blocks of 32
RPB = 4           # rows per block per core
F32 = mybir.dt.float32
BF16 = mybir.dt.bfloat16
BF16NP = ml_dtypes.bfloat16

SEG = 1088        # per-v slab columns: 2*(w1 + w2), w1 + w2 = 544
SLABW = RPB * SEG  # 4352
PACKW = 1092      # xt half (512) | w12 half (512) | xsel half (64) | b bits


def _rows_for_core(k: int) -> np.ndarray:
    """Row m = 4*b + v owns global row 32*b + 4*k + v."""
    b = np.repeat(np.arange(NB), RPB)
    v = np.tile(np.arange(RPB), NB)
    return 32 * b + RPB * k + v


# --------------------------------------------------------------------------
# device program
# --------------------------------------------------------------------------

_PROGRAM = None


def _build_program() -> bass.Bass:
    nc = bacc.Bacc()
    ADD = mybir.AluOpType.add

    # packed[h]: [xt[128h:128h+128] | w12t[128h:..] | xselt[128h:..] | bbits]
    # where bbits (h=0 only) = the two f32 bias columns viewed as 4 bf16.
    packed = nc.dram_tensor("packed", [2, 128, PACKW], BF16,
                            kind="ExternalInput")
    # slab[p]: blocks (p, 15-p); per v in [0,4): [A h0 (w1) | A h1 (w1) |
    # B h0 (w2) | B h1 (w2)] at offset 1088*v, w1 = 512-32p, w2 = 32+32p.
    slab = nc.dram_tensor("slab", [8, 128, SLABW], BF16,
                          kind="ExternalOutput")

    with tile.TileContext(nc) as tc:
        with (
            tc.tile_pool(name="const", bufs=1) as cpool,
            tc.tile_pool(name="psB", bufs=2, space="PSUM") as psB,
            tc.tile_pool(name="psR", bufs=2, space="PSUM") as psR,
            tc.tile_pool(name="slabs", bufs=4) as spool,
        ):
            # ---- input loads: two packed tiles, back-to-back on sync -----
            pk = []
            for h in range(2):
                t = cpool.tile([128, PACKW], BF16, tag=f"pk{h}")
                nc.sync.dma_start(out=t[:], in_=packed[h])
                pk.append(t)
            XT = [pk[h][:, 0:N] for h in range(2)]
            W12 = [pk[h][:, N:N + 2 * CH] for h in range(2)]
            XS = [pk[h][:, N + 2 * CH:N + 2 * CH + 64] for h in range(2)]
            bc = pk[0][:, 2 * N + 64:PACKW].bitcast(F32)  # [128, 2] f32

            mm = nc.tensor.matmul

            # ---- row terms RS[c, m] = A[row_m, c], two halves ------------
            RS = [None, None]    # f32, scalar operands for TS / ACT bias
            RS16 = [None, None]  # bf16, in1 for fused tensor_tensor
            for h in range(2):
                pr = psR.tile([128, 64], F32, tag="pr", name=f"pr{h}")
                lo = 128 * h  # W1.T columns
                mm(pr[:], W12[0][:, lo:lo + 128], XS[0],
                   start=True, stop=False)
                mm(pr[:], W12[1][:, lo:lo + 128], XS[1],
                   start=False, stop=True)
                rs = cpool.tile([128, 64], F32, tag=f"RS{h}")
                rs16 = cpool.tile([128, 64], BF16, tag=f"RS16{h}")
                nc.vector.tensor_copy(out=rs[:], in_=pr[:])
                nc.scalar.copy(out=rs16[:], in_=pr[:])
                RS[h] = rs
                RS16[h] = rs16

            # ---- column tables B'T[c, j] = B[j, c] + b[c], two halves ----
            BT = [None, None]
            for h in range(2):
                pb = psB.tile([128, N], F32, tag="pb", name=f"pb{h}")
                lo = CH + 128 * h  # W2.T columns
                mm(pb[:], W12[0][:, lo:lo + 128], XT[0],
                   start=True, stop=False)
                mm(pb[:], W12[1][:, lo:lo + 128], XT[1],
                   start=False, stop=True)
                bt = cpool.tile([128, N], BF16, tag=f"BT{h}")
                if h == 0:
                    nc.vector.tensor_scalar_add(bt[:], pb[:], bc[:, 0:1])
                else:
                    nc.scalar.add(bt[:], pb[:], bc[:, 1:2])
                BT[h] = bt

            def fused_B(eng, S, p, h, w1, w2, cB):
                """One op for block-B half h over all four v rows."""
                sfull = S[:]
                out = bass.AP(sfull.tensor, sfull.offset + 2 * w1 + h * w2,
                              [sfull.ap[0], [SEG, RPB], [1, w2]])
                btf = BT[h][:]
                in0 = bass.AP(btf.tensor, btf.offset + cB,
                              [btf.ap[0], [0, RPB], [1, w2]])
                rsf = RS16[h][:]
                in1 = bass.AP(rsf.tensor, rsf.offset + RPB * (15 - p),
                              [rsf.ap[0], [1, RPB], [0, w2]])
                eng.tensor_tensor(out=out, in0=in0, in1=in1, op=ADD)

            # ---- main loop: one slab per block pair ----------------------
            # p=0: v-major op order, half-slab DMAs, block-B on ACT per-v.
            # p>=1: DVE takes sA0 (+ sA1 for p>=5), ACT takes sA1 (p<=4)
            # and block-B for p>=6, GpSimd the fused block-B for p in 1..4,
            # DVE-fused for p=5.
            for p in range(8):
                w1 = N - 32 * p          # block p rect width (cols 32p..512)
                w2 = 32 + 32 * p         # block 15-p width
                cA = 32 * p              # B'T col offset for block p
                cB = N - w2              # for block 15-p
                S = spool.tile([128, SLABW], BF16, tag="s", name=f"s{p}")
                hp = tc.high_priority() if p == 0 else contextlib.nullcontext()
                with hp:
                    for v in range(RPB):
                        off = SEG * v
                        mA = RPB * p + v
                        mB = RPB * (15 - p) + v
                        sA0 = S[:, off:off + w1]
                        sA1 = S[:, off + w1:off + 2 * w1]
                        nc.vector.tensor_scalar_add(
                            sA0, BT[0][:, cA:N], RS[0][:, mA:mA + 1])
                        if p == 0 or p >= 5:
                            nc.vector.tensor_scalar_add(
                                sA1, BT[1][:, cA:N], RS[1][:, mA:mA + 1])
                        else:
                            nc.scalar.add(sA1, BT[1][:, cA:N],
                                          RS[1][:, mA:mA + 1])
                        if p == 0 or p >= 6:
                            for h in range(2):
                                sB = S[:, off + 2 * w1 + h * w2:
                                       off + 2 * w1 + (h + 1) * w2]
                                nc.scalar.add(sB, BT[h][:, cB:N],
                                              RS[h][:, mB:mB + 1])
                        if p == 0 and v == 1:
                            nc.sync.dma_start(out=slab[0][:, 0:2 * SEG],
                                              in_=S[:, 0:2 * SEG])
                    if 1 <= p <= 4:
                        fused_B(nc.gpsimd, S, p, 0, w1, w2, cB)
                        fused_B(nc.gpsimd, S, p, 1, w1, w2, cB)
                    elif p == 5:
                        fused_B(nc.vector, S, p, 0, w1, w2, cB)
                        fused_B(nc.vector, S, p, 1, w1, w2, cB)
                    if p == 0:
                        nc.sync.dma_start(out=slab[0][:, 2 * SEG:SLABW],
                                          in_=S[:, 2 * SEG:SLABW])
                    else:
                        nc.sync.dma_start(out=slab[p], in_=S[:])

    nc.compile()
    return nc


def _program() -> bass.Bass:
    global _PROGRAM
    if _PROGRAM is None:
        _PROGRAM = _build_program()
    return _PROGRAM


# --------------------------------------------------------------------------
# host entry point
# --------------------------------------------------------------------------

def build_in_maps(x, W, b):
    x = np.asarray(x, np.float32)
    W = np.asarray(W, np.float32)
    b = np.asarray(b, np.float32)
    w12 = np.concatenate(
        [np.ascontiguousarray(W[:, :NIN].T),
         np.ascontiguousarray(W[:, NIN:].T)], axis=1)  # [in, 512]
    xt = np.ascontiguousarray(x.T).astype(BF16NP)
    w12 = w12.astype(BF16NP)
    bbits = np.ascontiguousarray(
        np.stack([b[0:128], b[128:256]], axis=1)).view(BF16NP)  # [128, 4]
    maps = []
    for k in range(NCORES):
        rows = _rows_for_core(k)
        xsel = np.ascontiguousarray(x[rows].T).astype(BF16NP)
        packed = np.zeros((2, 128, PACKW), BF16NP)
        for h in range(2):
            lo = 128 * h
            packed[h, :, 0:N] = xt[lo:lo + 128]
            packed[h, :, N:N + 2 * CH] = w12[lo:lo + 128]
            packed[h, :, N + 2 * CH:N + 2 * CH + 64] = xsel[lo:lo + 128]
        packed[0, :, 2 * N + 64:PACKW] = bbits
        maps.append({"packed": packed})
    return maps


def _assemble(results):
    """8 per-core slab dicts -> full [512, 512, 256] f32 output."""
    out = np.zeros((N, N, CH), np.float32)
    ar = np.arange(RPB)
    for k in range(NCORES):
        slab = np.asarray(results[k]["slab"]).astype(np.float32)
        for p in range(8):
            w1 = N - 32 * p
            w2 = 32 + 32 * p
            sp = slab[p].reshape(128, RPB, SEG)
            rowsA = 32 * p + RPB * k + ar
            rowsB = 32 * (15 - p) + RPB * k + ar
            out[rowsA, 32 * p:N, 0:128] = \
                sp[:, :, 0:w1].transpose(1, 2, 0)
            out[rowsA, 32 * p:N, 128:256] = \
                sp[:, :, w1:2 * w1].transpose(1, 2, 0)
            out[rowsB, N - w2:N, 0:128] = \
                sp[:, :, 2 * w1:2 * w1 + w2].transpose(1, 2, 0)
            out[rowsB, N - w2:N, 128:256] = \
                sp[:, :, 2 * w1 + w2:SEG].transpose(1, 2, 0)
    # unshard: keep the strict upper triangle (sub-diagonal rect lanes are
    # garbage), mirror it across the diagonal; diag stays scatter-init 0.
    tril = np.tril_indices(N)
    out[tril] = 0.0
    return out + out.transpose(1, 0, 2)


def kernel(x, W, b):
    nc = _program()
    in_maps = build_in_maps(x, W, b)
    res = run_bass_kernel_spmd(nc, in_maps, core_ids=list(range(NCORES)))
    return _assemble(res.results)
